# revision 69
# baseline (speedup 1.0000x reference)
"""DynamicConv Trainium2 kernel.

Problem: x[32,256,64,64] f32. Attention branch (GAP -> FC(64) -> ReLU ->
FC(4) -> softmax) yields per-batch weights attn[b, k] over K=4 depthwise
3x3 kernels; output = sum_k attn[b,k] * depthwise_conv(x, kernel_k).

Algorithm used here (4x less conv work than the reference formulation):
the conv is linear in the kernel taps, so combine the K kernels first:
    w_eff[b,c,dy,dx] = sum_k attn[b,k] * conv_w[k,c,0,dy,dx]
then do ONE depthwise 3x3 conv per image with per-(b,c) taps.

Mapping (per NeuronCore, data-parallel over batch, 4 images/core):
  - channels on partitions (2 groups of 128), pixels on the free dim.
  - 7 of the 9 taps run on the TensorEngine as diagonal matmuls:
    lhsT = diag(w_eff[:,tap]) (float32r, 1 col/cycle), rhs = shifted view
    of the x tile; the 9-tap sum accumulates natively in PSUM per
    512-pixel bank. Diagonal matrices are built by bouncing w_eff through
    a pre-zeroed DRAM buffer (DRAM is linear => the diagonal is a single
    uniform-stride DMA).
  - center tap (0,0) runs on ScalarE (activation Copy, per-partition
    scale), tap (0,1) on VectorE (scalar_tensor_tensor fused MAC).
  - VectorE merges PSUM + the SBUF partial and writes the output tile
    (PSUM is not DMA-able).
  - GAP runs on ScalarE (Copy activation with accum_out), the attention
    MLP on PE (with fc2 bias folded into an augmented weight row), and
    the whole attention pipeline for batch b+1 is software-pipelined
    under batch b's conv.
"""

from contextlib import ExitStack

import numpy as np

B_FULL, C, H, W = 32, 256, 64, 64
K, KS, RED = 4, 3, 4
N_CORES = 8
B_LOC = B_FULL // N_CORES  # 4 images per core
NG = C // 128              # 2 channel groups of 128 partitions
HW = H * W                 # 4096 pixels
NBANKS = 8                 # 512-pixel PSUM banks per image
ROWS_PER_BANK = H // NBANKS  # 8 image rows per bank

TAPS = [(dy, dx) for dy in (-1, 0, 1) for dx in (-1, 0, 1)]
ACT_TAP = (0, 0)   # full coverage -> ScalarE write tap
DVE_TAP = (0, 1)   # VectorE fused-MAC tap (exact, no fixups)
# NOTE: the GpSimd/Pool engine does NOT support TensorScalarPtr ops on real
# TRN2 (walrus ISA check) — TimelineSim models them, but they cannot run.
# Pool is limited to memset/affine_select/partition_broadcast here.
PE_TAPS = [t for t in TAPS if t not in (ACT_TAP, DVE_TAP)]


def tap_idx(dy, dx):
    return (dy + 1) * 3 + (dx + 1)


def build_bass():
    import concourse.bacc as bacc
    import concourse.bass as bass
    import concourse.tile as tile
    from concourse import mybir

    f32 = mybir.dt.float32
    f32r = mybir.dt.float32r

    nc = bacc.Bacc("TRN2", target_bir_lowering=False)

    x_d = nc.dram_tensor("x", [B_LOC, C, H, W], f32, kind="ExternalInput")
    convw_d = nc.dram_tensor("conv_w", [K, C, 1, KS, KS], f32, kind="ExternalInput")
    fc1w_d = nc.dram_tensor("fc1_w", [C // RED, C], f32, kind="ExternalInput")
    fc1b_d = nc.dram_tensor("fc1_b", [C // RED], f32, kind="ExternalInput")
    fc2w_d = nc.dram_tensor("fc2_w", [K, C // RED], f32, kind="ExternalInput")
    fc2b_d = nc.dram_tensor("fc2_b", [K], f32, kind="ExternalInput")
    out_d = nc.dram_tensor("out", [B_LOC, C, H, W], f32, kind="ExternalOutput")

    DH = C // RED  # 64 hidden units

    with tile.TileContext(nc) as tc, ExitStack() as ctx:
        singles = ctx.enter_context(tc.tile_pool(name="singles", bufs=1))
        xin = ctx.enter_context(tc.tile_pool(name="xin", bufs=5))
        partials = ctx.enter_context(tc.tile_pool(name="partials", bufs=3))
        outs = ctx.enter_context(tc.tile_pool(name="outs", bufs=2))
        diags = ctx.enter_context(tc.tile_pool(name="diags", bufs=4))
        smalls = ctx.enter_context(tc.tile_pool(name="smalls", bufs=4))
        cpsum = ctx.enter_context(tc.tile_pool(name="cpsum", bufs=3, space="PSUM"))
        mpsum = ctx.enter_context(tc.tile_pool(name="mpsum", bufs=2, space="PSUM"))

        bf16 = mybir.dt.bfloat16
        NPT = len(PE_TAPS)  # 7
        gapscr = singles.tile([128, HW], f32, tag="gapscr")

        x_load_insts = {}

        def emit_load(b, pieces=1):
            x_t = []
            for g in range(NG):
                t = xin.tile([128, HW], f32r, tag="x", name=f"x_{b}_{g}")
                src_ap = x_d[b, g * 128:(g + 1) * 128, :, :].bitcast(f32r)
                src_fl = src_ap.rearrange("p h w -> p (h w)")
                if pieces > 1:
                    pz = HW // pieces
                    for q in range(pieces):
                        lo = q * pz
                        li = nc.sync.dma_start(out=t[:, lo:lo + pz],
                                               in_=src_fl[:, lo:lo + pz])
                        x_load_insts.setdefault(b, []).append(li)
                else:
                    li = nc.sync.dma_start(out=t[:], in_=src_ap)
                    x_load_insts.setdefault(b, []).append(li)
                x_t.append(t)
            return x_t

        # x(0) heads the DMA queue in quarter-pieces: the prologue GAP
        # reduces each piece as it lands, so the attention chain starts
        # ~1us after the last x(0) byte instead of after a full-tile reduce.
        # fc1_w is squeezed between the two groups' pieces — it feeds the
        # longest weight chain (PE transpose -> scale -> fc1wT).
        NP0 = 8
        QPX = HW // NP0
        x0 = [xin.tile([128, HW], f32r, tag="x", name=f"x_0_{g}")
              for g in range(NG)]
        fc1w_sb = singles.tile([DH, C], f32, tag="fc1w_sb")
        for q in range(NP0):
            for g in range(NG):
                li = nc.sync.dma_start(
                    out=x0[g][:, q * QPX:(q + 1) * QPX],
                    in_=x_d[0, g * 128:(g + 1) * 128, :, :].bitcast(f32r)
                    .rearrange("p h w -> p (h w)")[:, q * QPX:(q + 1) * QPX])
                x_load_insts.setdefault(0, []).append(li)
            if q == 1:
                nc.sync.dma_start(out=fc1w_sb[:], in_=fc1w_d[:])

        # ident128: [128,128] identity used to build diag(w) tiles ON-CHIP on
        # the (otherwise idle) GpSimd engine — the DRAM bounce this replaces
        # cost ~19us of DMA-engine occupancy plus a scatter->load roundtrip
        # on every batch's attention critical path.
        ones128 = singles.tile([128, 128], f32, tag="ones128")
        nc.gpsimd.memset(ones128[:], 1.0)
        ident128 = singles.tile([128, 128], f32, tag="ident128")
        nc.gpsimd.affine_select(
            out=ident128[:], in_=ones128[:], pattern=[[-1, 128]],
            compare_op=mybir.AluOpType.is_equal, fill=0.0,
            base=0, channel_multiplier=1)

        # ---- static weights -------------------------------------------------
        # fc2_wT augmented with a bias row: [h (partitions) + 1, k]; row DH
        # holds fc2_b, and h_aug = [relu(h); 1] folds the bias into the MM.
        fc2wT = singles.tile([DH + 1, K], f32, tag="fc2wT")
        nc.sync.dma_start(
            out=fc2wT[:DH, :],
            in_=bass.AP(tensor=fc2w_d, offset=0, ap=[[1, DH], [DH, K]]),
        )
        nc.sync.dma_start(out=fc2wT[DH:DH + 1, :],
                          in_=bass.AP(tensor=fc2b_d, offset=0,
                                      ap=[[K, 1], [1, K]]))

        fc1b = singles.tile([DH, 1], f32, tag="fc1b")
        nc.sync.dma_start(out=fc1b[:], in_=fc1b_d[:].unsqueeze(1))

        # all-ones [DH+1, 128] used to replicate the fc2 logits to every
        # partition via PE (see emit_attention: no partition_broadcast, so
        # the softmax tail never touches GpSimd's in-order queue).
        ones_dh = singles.tile([DH + 1, 128], f32, tag="ones_dh")
        nc.gpsimd.memset(ones_dh[:], 1.0)

        # conv_w per grp: [c (partitions), k, 9 taps] in ONE DMA per group
        # (each extra DMA costs ~650ns of serialized issue in the prologue).
        KK = KS * KS
        convw_sb = [[None] * K for _ in range(NG)]
        for g in range(NG):
            t = singles.tile([128, K * KK], f32, tag=f"cw{g}")
            src = bass.AP(tensor=convw_d,
                          offset=g * 128 * KK,
                          ap=[[KK, 128], [C * KK, K], [1, KK]])
            nc.sync.dma_start(out=t[:], in_=src)
            for k in range(K):
                convw_sb[g][k] = t[:, k * KK:(k + 1) * KK]

        # fc1_wT[grp]: [c within group (partitions), m] = fc1_w[m, c] / HW
        # (the 1/HW folds the GAP mean into fc1; fc1_b is added after, so
        #  this matches relu(mean(x) @ fc1_w.T + fc1_b)).
        # fc1_w is loaded CONTIGUOUSLY (a strided 4-byte transpose-gather
        # DMA costs ~3.6us and gates the whole prologue attention chain)
        # and transposed on-chip via PE with an affine_select identity.
        fc1wT = []
        for g in range(NG):
            tps = mpsum.tile([128, DH], f32, tag="mlp")
            nc.tensor.transpose(tps[:], fc1w_sb[:, g * 128:(g + 1) * 128],
                                ident128[:DH, :DH])
            t = singles.tile([128, DH], f32, tag=f"fc1wT{g}")
            nc.scalar.mul(t[:], tps[:], 1.0 / HW)
            fc1wT.append(t)

        def emit_gap(b, x_t):
            """Per-channel spatial sums for batch b.

            On ScalarE (the Copy activation's accum_out yields the sum) so
            VectorE stays clear for conv merges. The prologue batch reduces
            quarter-pieces on DVE (g0) and ACT (g1) as each piece's DMA
            lands, so gsum(0) is ready ~1us after the last x(0) byte.
            """
            gsum = smalls.tile([128, NG], f32, tag="gsum", name=f"gsum_{b}")
            if b == 0:
                gs16 = smalls.tile([128, 16], f32, tag="gs16")
                QP = HW // NP0
                for q in range(NP0):
                    nc.vector.tensor_reduce(
                        out=gs16[:, q:q + 1],
                        in_=x_t[0][:, q * QP:(q + 1) * QP].bitcast(f32),
                        axis=mybir.AxisListType.X, op=mybir.AluOpType.add)
                    # g1: first half on ACT, later pieces on DVE (free by
                    # then), so the last-landing piece reduces immediately
                    if q < NP0 // 2:
                        nc.scalar.activation(gapscr[:, 0:QP],
                                             x_t[1][:, q * QP:(q + 1) * QP]
                                             .bitcast(f32),
                                             mybir.ActivationFunctionType.Copy,
                                             bias=0.0, scale=1.0,
                                             accum_out=gs16[:, 8 + q:9 + q])
                    else:
                        nc.vector.tensor_reduce(
                            out=gs16[:, 8 + q:9 + q],
                            in_=x_t[1][:, q * QP:(q + 1) * QP].bitcast(f32),
                            axis=mybir.AxisListType.X, op=mybir.AluOpType.add)
                for g in range(NG):
                    nc.vector.tensor_reduce(
                        out=gsum[:, g:g + 1],
                        in_=gs16[:, 8 * g:8 * g + NP0],
                        axis=mybir.AxisListType.X, op=mybir.AluOpType.add)
            else:
                for g in range(NG):
                    nc.scalar.activation(gapscr[:], x_t[g][:].bitcast(f32),
                                         mybir.ActivationFunctionType.Copy,
                                         bias=0.0, scale=1.0,
                                         accum_out=gsum[:, g:g + 1])
            return gsum

        def make_attention(b, gsum):
            """MLP -> softmax -> w_eff -> diag tiles for batch b, as three
            emitter stages so the pipeline can interleave them into the
            PREVIOUS batch's conv emission (the PE matmuls land mid-stream
            and every cross-engine hop overlaps conv work).

            The softmax tail is broadcast-free: fc2wT is scaled by h on DVE
            (per-partition scalar), then matmul(ones[DH+1,128], fc2wT*h)
            replicates the logits to all 128 partitions in PSUM; the Exp's
            accum_out yields the softmax denominator on every partition.
            """
            st = {}

            def stage_mlp():
                h_ps = mpsum.tile([DH, 1], f32, tag="mlp")
                for g in range(NG):
                    nc.tensor.matmul(h_ps[:], fc1wT[g][:], gsum[:, g:g + 1],
                                     start=(g == 0), stop=(g == NG - 1))
                # h_aug = [relu(h); 1] folds fc2_b into the logits matmul.
                h_sb = smalls.tile([DH + 1, 1], f32, tag="h_sb")
                nc.scalar.activation(h_sb[:DH], h_ps[:],
                                     mybir.ActivationFunctionType.Relu,
                                     bias=fc1b[:], scale=1.0)
                nc.vector.memset(h_sb[DH:DH + 1, :], 1.0)
                hw_t = smalls.tile([DH + 1, K], f32, tag="hw_t")
                nc.vector.tensor_scalar_mul(hw_t[:], fc2wT[:], h_sb[:, 0:1])
                st["hw_t"] = hw_t

            def stage_logits():
                a_ps = mpsum.tile([128, K], f32, tag="mlp")
                nc.tensor.matmul(a_ps[:], ones_dh[:], st["hw_t"][:],
                                 start=True, stop=True)
                # softmax without the max-subtraction: the logits are O(1)
                # (h and fc2_w are small), so exp cannot overflow and
                # exp(x)/sum(exp(x)) matches the reference softmax exactly.
                e_bc = smalls.tile([128, K], f32, tag="e_bc")
                ssum = smalls.tile([128, 1], f32, tag="ssum")
                nc.scalar.activation(e_bc[:], a_ps[:],
                                     mybir.ActivationFunctionType.Exp,
                                     bias=0.0, scale=1.0, accum_out=ssum[:])
                r_bc = smalls.tile([128, 1], f32, tag="r_bc")
                nc.vector.reciprocal(r_bc[:], ssum[:])
                st["e_bc"], st["r_bc"] = e_bc, r_bc

            def stage_weff():
                e_bc, r_bc = st["e_bc"], st["r_bc"]
                weff, negw = [], []
                for g in range(NG):
                    wt = smalls.tile([128, KS * KS], f32, tag=f"weff{g}")
                    nc.vector.tensor_scalar_mul(wt[:], convw_sb[g][0],
                                                e_bc[:, 0:1])
                    for k in range(1, K):
                        nc.vector.scalar_tensor_tensor(
                            out=wt[:], in0=convw_sb[g][k],
                            scalar=e_bc[:, k:k + 1], in1=wt[:],
                            op0=mybir.AluOpType.mult, op1=mybir.AluOpType.add)
                    nc.vector.tensor_scalar_mul(wt[:], wt[:], r_bc[:])
                    weff.append(wt)
                    nt = smalls.tile([128, KS * KS], f32, tag=f"negw{g}")
                    nc.vector.tensor_scalar_mul(nt[:], wt[:], -1.0)
                    negw.append(nt)
                st["wn"] = (weff, negw)

            def stage_diag():
                # diag(w_eff[:, tap]) per PE tap, split across ScalarE and
                # GpSimd (plus VectorE for the latency-critical prologue
                # batch) so the first group's diags are ready right as the
                # PE finishes the previous batch.
                weff, negw = st["wn"]
                diag_sb = []
                order = PE_TAPS
                if b == 0:
                    # prologue: build the pair-0/other lead taps first so
                    # the first conv matmul unblocks as early as possible
                    order = [(1, 1), (-1, 0)] + \
                        [t for t in PE_TAPS if t not in ((1, 1), (-1, 0))]
                for g in range(NG):
                    wt = weff[g]
                    dt_ = diags.tile([128, NPT, 128], f32r, tag="diag",
                                     name=f"diag_{b}_{g}")
                    for i, (dy, dx) in enumerate(order):
                        s = PE_TAPS.index((dy, dx))
                        col = tap_idx(dy, dx)
                        # ScalarE (292ns) and VectorE (127ns, 2x mode) only:
                        # GpSimd cannot host TensorScalarPtr on HW. DVE gets
                        # the larger share (it is cheaper per slot).
                        if i % 3 == 0:
                            nc.scalar.activation(
                                dt_[:, s, :], ident128[:],
                                mybir.ActivationFunctionType.Copy,
                                bias=0.0, scale=wt[:, col:col + 1])
                        else:
                            nc.vector.tensor_scalar_mul(
                                dt_[:, s, :], ident128[:], wt[:, col:col + 1])
                    diag_sb.append(dt_)
                st["res"] = (weff, negw, diag_sb)

            return st, [stage_mlp, stage_logits, stage_weff, stage_diag]

        def emit_part_one(b, g, x_t, weff, negw, nhalves=1):
            """SBUF partial for one group: ScalarE center tap + DVE edge
            fix-ups.

            The PE taps run on the FLAT image: out_flat[i] += w*x_flat[i+S],
            S = 64*dy + dx (fully contiguous => legal 2D matmul APs), with
            ranges rounded inward to even boundaries (f32r matmuls need even
            PSUM offset/size). The partial pre-subtracts the row-wrap terms
            the flat MMs wrongly add at one edge column per row, and adds
            back the 1-2 true edge contributions the even rounding dropped,
            so the PSUM+partial merge yields the exact zero-padded conv.
            """
            xt = x_t[g][:].bitcast(f32)
            x3 = xt.rearrange("p (h w) -> p h w", w=W)
            part = partials.tile([128, HW], f32, tag="part",
                                 name=f"part_{b}_{g}")
            p3 = part[:].rearrange("p (h w) -> p h w", w=W)
            # center tap on ScalarE: part = x * w[4] (optionally in halves so
            # later pieces land just-in-time while the GAP copies for the
            # next batch run earlier on ScalarE's in-order queue)
            hz = HW // nhalves
            for i in range(nhalves):
                if i == nhalves - 1 and nhalves > 1:
                    # last half on VectorE's 2x tensor_scalar path: shortens
                    # the ScalarE chain that gates the group's tap ops
                    nc.vector.tensor_scalar_mul(part[:, i * hz:(i + 1) * hz],
                                                xt[:, i * hz:(i + 1) * hz],
                                                weff[g][:, 4:5])
                else:
                    nc.scalar.activation(part[:, i * hz:(i + 1) * hz],
                                         xt[:, i * hz:(i + 1) * hz],
                                         mybir.ActivationFunctionType.Copy,
                                         bias=0.0, scale=weff[g][:, 4:5])
            for (dy, dx) in PE_TAPS:
                if dx == 0:
                    continue
                ti = tap_idx(dy, dx)
                S = W * dy + dx
                i0 = max(0, -S)
                i1 = HW - max(0, S)
                i0e = (i0 + 1) // 2 * 2
                i1e = i1 // 2 * 2
                # subtract row-wrap terms: p = 64h + e in [i0e, i1e)
                e = 0 if dx < 0 else W - 1
                s = W - 1 - e
                d = dy + dx
                h0 = -(-(i0e - e) // W)       # ceil div
                h1 = (i1e - 1 - e) // W + 1
                nc.vector.scalar_tensor_tensor(
                    out=p3[:, h0:h1, e:e + 1],
                    in0=x3[:, h0 + d:h1 + d, s:s + 1],
                    scalar=negw[g][:, ti:ti + 1],
                    in1=p3[:, h0:h1, e:e + 1],
                    op0=mybir.AluOpType.mult, op1=mybir.AluOpType.add)
                # add back dropped true contributions
                for p in ([i0] if i0e > i0 else []) + \
                         ([i1e] if i1 > i1e else []):
                    h, w_ = divmod(p, W)
                    if 0 <= h + dy < H and 0 <= w_ + dx < W:
                        nc.vector.scalar_tensor_tensor(
                            out=p3[:, h:h + 1, w_:w_ + 1],
                            in0=x3[:, h + dy:h + dy + 1,
                                   w_ + dx:w_ + dx + 1],
                            scalar=weff[g][:, ti:ti + 1],
                            in1=p3[:, h:h + 1, w_:w_ + 1],
                            op0=mybir.AluOpType.mult,
                            op1=mybir.AluOpType.add)
            return part

        PAIR = 2                   # PSUM banks per accumulation tile
        PPX = PAIR * 512           # 1024 pixels per pair
        PROWS = PPX // W           # 16 image rows per pair
        NPAIR = NBANKS // PAIR     # 4 pairs per group-image

        def emit_conv_stage(b, x_t, weff, diag_sb, parts, groups, hooks=None):
            hooks = hooks or {}
            for g in groups:
                xr = x_t[g][:]                      # [128, HW] float32r
                x3 = xr.bitcast(f32).rearrange("p (h w) -> p h w", w=W)
                part = parts[g]
                p3 = part[:].rearrange("p (h w) -> p h w", w=W)

                out_t = outs.tile([128, HW], f32, tag="out",
                                  name=f"out_{b}_{g}")
                o3 = out_t[:].rearrange("p (h w) -> p h w", w=W)
                last_grp = (b == B_LOC - 1 and g == NG - 1)
                for pr in range(NPAIR):
                    for fn in hooks.get((g, pr), []):
                        fn()
                    pb0 = pr * PPX
                    r0 = pr * PROWS
                    r1 = min(r0 + PROWS, H - 1)
                    # (0,1) tap on VectorE per pair (exact ranges, no
                    # fixups; GpSimd cannot host TensorScalarPtr on HW)
                    nc.vector.scalar_tensor_tensor(
                        out=p3[:, r0:r0 + PROWS, 0:W - 1],
                        in0=x3[:, r0:r0 + PROWS, 1:W],
                        scalar=weff[g][:, 5:6],
                        in1=p3[:, r0:r0 + PROWS, 0:W - 1],
                        op0=mybir.AluOpType.mult,
                        op1=mybir.AluOpType.add)

                    ps = cpsum.tile([128, PPX], f32, tag="cps",
                                    name=f"cps_{b}_{g}_{pr}")
                    # tap-major within the pair: the same diag slot feeds
                    # consecutive matmuls, so walrus emits one LDWEIGHTS per
                    # tap instead of one per matmul. The first tap is
                    # full-coverage for every bank in the pair so it sets
                    # has_written across each bank.
                    first = (1, 0) if pr == 0 else (-1, 0)
                    taps = [first] + [t for t in PE_TAPS if t != first]
                    for i, (dy, dx) in enumerate(taps):
                        S = W * dy + dx
                        t0 = max(0, -S)
                        t1 = HW - max(0, S)
                        for j in range(pr * PAIR, (pr + 1) * PAIR):
                            b0 = j * 512
                            i0 = max(b0, (t0 + 1) // 2 * 2)
                            i1 = min(b0 + 512, t1 // 2 * 2)
                            nc.tensor.matmul(
                                ps[:, i0 - pb0:i1 - pb0],
                                diag_sb[g][:, PE_TAPS.index((dy, dx)), :],
                                xr[:, i0 + S:i1 + S],
                                start=(i == 0), stop=(i == len(taps) - 1),
                                skip_group_check=True)
                    out_fl = out_d[b, g * 128:(g + 1) * 128, :, :] \
                        .rearrange("p h w -> p (h w)")
                    final = (b == B_LOC - 1 and g == NG - 1
                             and pr >= NPAIR - 2)
                    if final:
                        # epilogue tail: nothing overlaps it, so merge and
                        # store per BANK (each store issued from its own
                        # engine queue) to shorten the serial tail.
                        for half, eng in ((0, nc.sync), (1, nc.scalar)):
                            plo = pb0 + half * 512
                            nc.vector.tensor_add(out_t[:, plo:plo + 512],
                                                 ps[:, half * 512:
                                                     half * 512 + 512],
                                                 part[:, plo:plo + 512])
                            eng.dma_start(out=out_fl[:, plo:plo + 512],
                                          in_=out_t[:, plo:plo + 512])
                    else:
                        # merge PSUM + SBUF partial -> out (drains PSUM),
                        # one VectorE op per pair
                        nc.vector.tensor_add(out_t[:, pb0:pb0 + PPX],
                                             ps[:], part[:, pb0:pb0 + PPX])
                        # drain every 2 finished pairs to DRAM
                        if pr % 2 == 1:
                            lo = (pr - 1) * PPX
                            nc.sync.dma_start(
                                out=out_fl[:, lo:lo + 2 * PPX],
                                in_=out_t[:, lo:lo + 2 * PPX])

        # Software pipeline, one batch ahead. Per-engine program order per
        # iteration b:
        #   ScalarE: center(b) g0 -> GAP(b+1) g0,g1 -> center(b) g1 (halves,
        #            just-in-time for g1's merges) -> relu -> exp -> diag
        #   VectorE: fixups(b) g0 -> g0 row-taps+merges -> fixups g1 ->
        #            g1 row-taps+merges with the attention hops interleaved
        #   PE:      conv(b) g0 -> conv g1 with the two MLP matmuls landing
        #            mid-stream (their cross-engine deps overlap conv runway)
        #   GpSimd:  (0,1)/row taps g0, g1 -> its diag half
        # so the attention chain for b+1 completes ~2us before batch b ends
        # and the PE never waits on diag tiles at a batch boundary.
        x_tiles = {0: x0}

        # prologue: attention(0) runs serially (nothing to hide it under).
        st0, stages0 = make_attention(0, emit_gap(0, x_tiles[0]))
        for fn in stages0:
            fn()
        stages = {0: st0["res"]}
        if B_LOC > 1:
            x_tiles[1] = emit_load(1)

        # Attention is pipelined TWO batches ahead: gap(X) is emitted at the
        # end of iteration X-2 (its x tiles land mid-batch X-2), and the
        # MLP->softmax->w_eff->diag stages ride iteration X-1's group-0 conv
        # hooks — a full batch after the GAP, so every stage's deps are
        # long-ready when its engine reaches it (no mid-conv dependency
        # stalls), yet the diags still complete ~a full period before use.
        # (Batch 1's attention rides batch 0's group-1 hooks to bootstrap.)
        gsums = {}
        if B_LOC > 1:
            gsums[1] = emit_gap(1, x_tiles[1])

        for b in range(B_LOC):
            x_t = x_tiles.pop(b)
            weff, negw, diag_sb = stages.pop(b)
            parts = [None, None]
            parts[0] = emit_part_one(b, 0, x_t, weff, negw, nhalves=1)
            hooks0, st_1, fns_1 = {}, None, None
            if b == 0 and B_LOC > 1:
                st_1, fns_1 = make_attention(1, gsums.pop(1))
                hooks0 = {(0, 1): [fns_1[0]], (0, 2): [fns_1[1]],
                          (0, 3): [fns_1[2]]}
            emit_conv_stage(b, x_t, weff, diag_sb, parts, groups=[0],
                            hooks=hooks0)
            if st_1 is not None:
                fns_1[3]()
                stages[1] = st_1["res"]
            parts[1] = emit_part_one(b, 1, x_t, weff, negw, nhalves=2)
            hooks1, st_n, fns_n = {}, None, None
            if b + 2 < B_LOC:
                x_tiles[b + 2] = emit_load(b + 2)
                # gap(b+2) runs on ScalarE whenever it is idle — bump its
                # priority far later so the scheduler never tie-breaks it
                # ahead of batch b+1's center taps (which gate merges).
                prio = tc.cur_priority
                tc.cur_priority = prio + 1500
                gsums[b + 2] = emit_gap(b + 2, x_tiles[b + 2])
                tc.cur_priority = prio
                st_n, fns_n = make_attention(b + 2, gsums.pop(b + 2))
                hooks1 = {(1, 1): [fns_n[0]], (1, 2): [fns_n[1]],
                          (1, 3): [fns_n[2]]}
            emit_conv_stage(b, x_t, weff, diag_sb, parts, groups=[1],
                            hooks=hooks1)
            if st_n is not None:
                fns_n[3]()
                stages[b + 2] = st_n["res"]

    nc.compile()
    return nc


_COMPILED = None
LAST_RESULTS = None


def kernel(**inputs):
    global _COMPILED
    import concourse.mybir as mybir  # noqa: F401  (import side effects)
    from concourse.bass_utils import run_bass_kernel_spmd

    if _COMPILED is None:
        _COMPILED = build_bass()
    nc = _COMPILED

    x = np.ascontiguousarray(inputs["x"], dtype=np.float32)
    rep = {k: np.ascontiguousarray(v, dtype=np.float32)
           for k, v in inputs.items() if k != "x"}

    in_maps = []
    for i in range(N_CORES):
        m = {"x": np.ascontiguousarray(x[i * B_LOC:(i + 1) * B_LOC])}
        m.update(rep)
        in_maps.append(m)

    # the axon-tunneled PJRT execute can fail transiently; retry a couple
    # of times before giving up.
    last_exc = None
    for attempt in range(3):
        try:
            res = run_bass_kernel_spmd(nc, in_maps,
                                       core_ids=list(range(N_CORES)))
            break
        except Exception as e:  # noqa: BLE001
            last_exc = e
            import time
            time.sleep(2.0 * (attempt + 1))
    else:
        raise last_exc
    global LAST_RESULTS
    LAST_RESULTS = res
    return np.concatenate([r["out"] for r in res.results], axis=0)

